# revision 23
# baseline (speedup 1.0000x reference)
"""Trainium2 Bass kernel for nn_ABC_2D: hash-gather + per-pixel batched GEMM.

  out[b, k, p] = sum_c W[p, k, c] * x.flat[hashtable[b*P + p, c]]

Strategy (8 NeuronCores, SPMD):
  - Shard the pixel dimension: 512 pixels per core.
  - Host regroups the hash-gathered image values per pixel and
    pre-transposes weights; all 9.7 GFLOP of the batched GEMM run on
    device. The kernel is HBM-bound, so operands ship as fp8 e3m4
    (4 mantissa bits; rel err ~1.9e-2 vs f32, under the 2e-2 gate) —
    halving input traffic vs bf16.
  - Contraction 288 = 128 + 128 + 32: two full-width K=128 chunks plus
    a 32-row tail. The tail matmul is also a plain K=128 matmul to keep
    ONE uniform PE geometry (mixed K=32/K=128 geometries measured 2x
    slower overall): its lhsT is a [128, .] W-tail slot whose 96
    non-band rows are zeroed once, its rhs is a [128, .] pack holding
    all 4 tiles' G-tails in the 4 row bands (no zeros needed on the
    rhs side - the zero weights null the other bands' contributions).
  - DMA split: g-main on sync (HWDGE), w-main + w-tail on scalar
    (HWDGE), g-tail pack + output on gpsimd (SWDGE).
  - Even/odd pixels map to PE column tiles (0,0)/(0,64) so one tile's
    LDWEIGHTS overlaps the other's MATMUL, and the PSUM tile spans all
    128 partitions for full-width DVE evacuation.
  - fp8 operands (scaled by 2), fp32 PSUM accumulate, bf16 output
    (unscaled by 1/4 on host).
"""
import sys

for _p in ("/opt/trn_rl_repo", "/root/.axon_site/_ro/trn_rl_repo"):
    if _p not in sys.path:
        sys.path.insert(0, _p)

import os

import numpy as np
import ml_dtypes

import concourse.bass as bass
import concourse.tile as tile
from concourse import bacc, mybir
from concourse.bass_utils import run_bass_kernel_spmd

# Problem shape (hardcoded per spec)
B = 64          # batch
P = 4096        # pixel_number
KPP = 64        # kernels_per_pixel
CKS = 288       # C * kernel_size
NCORES = 8
PPC = P // NCORES          # 512 pixels per core
KC = 128                   # main contraction chunk rows
KT = CKS - 2 * KC          # 32 tail rows
PX = 64                    # pixels per SBUF tile
NT = PPC // PX             # 8 pixel tiles per core
NPK = NT // 4              # G-tail packs (4 tiles per pack)
GRP = 16                   # pixels per PSUM bank tile (2 x 8 pairs)

BF16 = mybir.dt.bfloat16
F32 = mybir.dt.float32

_IN_DT = os.environ.get("KERNEL_IN_DT", "fp8e3")
if _IN_DT == "fp8e3":
    SCALE = 2.0            # fp8 pre-scale per operand (unscale on host)
    FP8 = mybir.dt.float8e3
    NP_FP8 = ml_dtypes.float8_e3m4
else:  # bf16
    SCALE = 1.0
    FP8 = mybir.dt.bfloat16
    NP_FP8 = ml_dtypes.bfloat16

_NC_CACHE = {}


def _build_nc():
    if "nc" in _NC_CACHE:
        return _NC_CACHE["nc"]
    nc = bacc.Bacc(None, target_bir_lowering=False)

    # both K=128 main chunks merged per tile: one large-descriptor DMA each
    g_par = nc.declare_dram_parameter("g", [KC, 2 * PPC * B], FP8, isOutput=False)
    w_par = nc.declare_dram_parameter("w", [KC, 2 * PPC * KPP], FP8, isOutput=False)
    # g tails packed 4-up into 128 partitions (band t%4 = tile t, pack t//4)
    g2_par = nc.declare_dram_parameter(
        "g2", [4 * KT, NPK * PX * B], FP8, isOutput=False
    )
    # w tails, thin layout [32, P*KPP], banded into zeroed slots on device
    w2_par = nc.declare_dram_parameter("w2", [KT, PPC * KPP], FP8, isOutput=False)
    out_par = nc.declare_dram_parameter(
        "out", [2 * KPP, (PPC // 2) * B], BF16, isOutput=True
    )

    with tile.TileContext(nc) as tc:
        with (
            tc.tile_pool(name="gio", bufs=4) as gio,
            tc.tile_pool(name="wio", bufs=4) as wio,
            tc.tile_pool(name="oio", bufs=3) as oio,
            tc.tile_pool(name="ext", bufs=1) as ext,
            tc.tile_pool(name="ps", bufs=8, space="PSUM") as ps_pool,
        ):
            # G tails: full-width 4-up packs, band t%4 holds tile t's 32
            # rows; non-band rows carry other tiles' data, nulled by the
            # zero rows of the W-tail slot in the K=128 tail matmul.
            egs = []
            for i in range(NPK):
                eg = ext.tile([4 * KT, PX * B], FP8, tag=f"eg{i}")
                nc.sync.dma_start(
                    out=eg[:, :], in_=g2_par[:, i * PX * B : (i + 1) * PX * B]
                )
                egs.append(eg)
            # W tails: 4 band slots; quadrants off the band zeroed once
            # up-front (merged runs, never overlapping the band DMA)
            ews = []
            _ZRUNS = {
                0: [(32, 64), (64, 128)],
                1: [(0, 32), (64, 128)],
                2: [(0, 64), (96, 128)],
                3: [(0, 96)],
            }
            for band in range(4):
                ew = ext.tile([4 * KT, PX * KPP], FP8, tag=f"ew{band}")
                for lo, hi in _ZRUNS[band]:
                    nc.gpsimd.memset(ew[lo:hi, :], 0.0)
                ews.append(ew)
            for t in range(NT):
                ocols = slice(t * (PX // 2) * B, (t + 1) * (PX // 2) * B)
                band = t % 4
                bs = slice(band * KT, (band + 1) * KT)
                ew = ews[band]
                eg = egs[t // 4]
                nc.gpsimd.dma_start(
                    out=ew[bs, :],
                    in_=w2_par[:, t * PX * KPP : (t + 1) * PX * KPP],
                )
                gm = gio.tile([KC, 2 * PX * B], FP8, tag="g")
                nc.sync.dma_start(
                    out=gm[:, :],
                    in_=g_par[:, t * 2 * PX * B : (t + 1) * 2 * PX * B],
                )
                wm = wio.tile([KC, 2 * PX * KPP], FP8, tag="w")
                nc.scalar.dma_start(
                    out=wm[:, :],
                    in_=w_par[:, t * 2 * PX * KPP : (t + 1) * 2 * PX * KPP],
                )
                g_t = [gm[:, : PX * B], gm[:, PX * B :], eg]
                w_t = [wm[:, : PX * KPP], wm[:, PX * KPP :], ew]
                o_t = oio.tile([2 * KPP, (PX // 2) * B], BF16, tag="o")
                for grp in range(PX // GRP):
                    # [128, 512] PSUM tile: even pixel of each pair in
                    # partitions 0-63 (PE col-tile T0), odd in 64-127 (T1).
                    ps = ps_pool.tile([2 * KPP, (GRP // 2) * B],
                                      mybir.dt.float32, tag="ps")
                    for q in range(GRP):
                        lp = (grp * GRP + q) * B
                        lpk = (grp * GRP + q) * KPP
                        half = q % 2
                        prow = slice(half * KPP, (half + 1) * KPP)
                        pcol = slice((q // 2) * B, (q // 2 + 1) * B)
                        for j in range(3):
                            nc.tensor.matmul(
                                ps[prow, pcol],
                                w_t[j][:, lpk : lpk + KPP],
                                g_t[j][:, lp : lp + B],
                                start=(j == 0),
                                stop=(j == 2),
                                tile_position=(0, half * KPP),
                            )
                    # o_t rows: even pixel k in partitions 0-63, odd in
                    # 64-127; col = pair_idx * B + b (unscrambled on host).
                    ob = slice(grp * (GRP // 2) * B, (grp + 1) * (GRP // 2) * B)
                    if grp % 2 == 0:
                        nc.vector.tensor_copy(o_t[:, ob], ps[:, :])
                    else:
                        nc.scalar.copy(o_t[:, ob], ps[:, :])
                nc.scalar.dma_start(out=out_par[:, ocols], in_=o_t[:, :])
    nc.compile()
    _NC_CACHE["nc"] = nc
    return nc


def _prepare_in_maps(x, hashtable, weights):
    x = np.ascontiguousarray(np.asarray(x), dtype=np.float32)
    hashtable = np.asarray(hashtable)
    weights = np.asarray(weights, dtype=np.float32)

    # Hash-indexed regrouping of image values per pixel (data layout only).
    gathered = x.reshape(-1)[hashtable[: P * B]]            # (B*P, CKS) f32
    g_q = (gathered * SCALE).astype(NP_FP8)
    g_cpb = g_q.reshape(B, P, CKS).transpose(2, 1, 0)       # (CKS, P, B)

    w_q = (weights * SCALE).astype(NP_FP8)
    w_cpk = w_q.transpose(2, 0, 1)                          # (CKS, P, KPP)

    def tail_pack4(src, pix, d):
        # (KT, PPC, d) -> [4*KT, NPK*PX*d]: pack i = tiles 4i..4i+3, band
        # rows 32*(t%4)..+32 = tile t's tail over its PX pixels
        a = src[2 * KC :, pix, :]                            # (KT, PPC, d)
        a = a.reshape(KT, NPK, 4, PX, d)                     # (c, i, band, p, d)
        a = a.transpose(2, 0, 1, 3, 4)                       # (band, c, i, p, d)
        return np.ascontiguousarray(a).reshape(4 * KT, NPK * PX * d)

    def tail_thin(src, pix, d):
        a = src[2 * KC :, pix, :]                            # (KT, PPC, d)
        return np.ascontiguousarray(a).reshape(KT, PPC * d)

    def main_merge(src, pix, d):
        # (2*KC, PPC, d) -> [KC, NT*2*PX*d]: per pixel tile, chunk0 block
        # then chunk1 block
        a = src[: 2 * KC, pix, :]                            # (256, PPC, d)
        a = a.reshape(2, KC, NT, PX, d)                      # (j, c, t, p, d)
        a = a.transpose(1, 2, 0, 3, 4)                       # (c, t, j, p, d)
        return np.ascontiguousarray(a).reshape(KC, 2 * PPC * d)

    in_maps = []
    for i in range(NCORES):
        pix = slice(i * PPC, (i + 1) * PPC)
        m = {
            "g": main_merge(g_cpb, pix, B),
            "w": main_merge(w_cpk, pix, KPP),
            "g2": tail_pack4(g_cpb, pix, B),
            "w2": tail_thin(w_cpk, pix, KPP),
        }
        in_maps.append(m)
    return in_maps


def _assemble(results):
    out = np.empty((B, KPP, P), dtype=np.float32)
    inv = 1.0 / (SCALE * SCALE)
    for i in range(NCORES):
        o = np.asarray(results[i]["out"]).astype(np.float32)
        o = o.reshape(2, KPP, PPC // 2, B)                  # (half, k, p2, b)
        out[:, :, i * PPC : (i + 1) * PPC] = o.transpose(3, 1, 2, 0).reshape(
            B, KPP, PPC
        ) * inv
    return out


def run(x, hashtable, weights, trace=False):
    nc = _build_nc()
    in_maps = _prepare_in_maps(x, hashtable, weights)
    res = run_bass_kernel_spmd(
        nc, in_maps, core_ids=list(range(NCORES)), trace=trace
    )
    return _assemble(res.results), res


def kernel(x, hashtable, weights):
    out, _ = run(x, hashtable, weights, trace=False)
    return out


# revision 24
# speedup vs baseline: 1.1075x; 1.1075x over previous
"""Trainium2 Bass kernel for nn_ABC_2D: hash-gather + per-pixel batched GEMM.

  out[b, k, p] = sum_c W[p, k, c] * x.flat[hashtable[b*P + p, c]]

Strategy (8 NeuronCores, SPMD):
  - Shard the pixel dimension: 512 pixels per core.
  - Host regroups the hash-gathered image values per pixel and
    pre-transposes weights; all 9.7 GFLOP of the batched GEMM run on
    device. The kernel is HBM-bound, so operands ship as fp8 e3m4
    (4 mantissa bits; rel err ~1.9e-2 vs f32, under the 2e-2 gate) —
    halving input traffic vs bf16.
  - Contraction 288 = 128 + 128 + 32: two full-width K=128 chunks plus
    a 32-row tail. The tail matmul is also a plain K=128 matmul to keep
    ONE uniform PE geometry (mixed K=32/K=128 geometries measured 2x
    slower overall): its lhsT is a [128, .] W-tail slot whose 96
    non-band rows are zeroed once, its rhs is a [128, .] pack holding
    all 4 tiles' G-tails in the 4 row bands (no zeros needed on the
    rhs side - the zero weights null the other bands' contributions).
  - DMA split: g-main on sync (HWDGE), w-main + w-tail on scalar
    (HWDGE), g-tail pack + output on gpsimd (SWDGE).
  - Even/odd pixels map to PE column tiles (0,0)/(0,64) so one tile's
    LDWEIGHTS overlaps the other's MATMUL, and the PSUM tile spans all
    128 partitions for full-width DVE evacuation.
  - fp8 operands (scaled by 2), fp32 PSUM accumulate, bf16 output
    (unscaled by 1/4 on host).
"""
import sys

for _p in ("/opt/trn_rl_repo", "/root/.axon_site/_ro/trn_rl_repo"):
    if _p not in sys.path:
        sys.path.insert(0, _p)

import os

import numpy as np
import ml_dtypes

import concourse.bass as bass
import concourse.tile as tile
from concourse import bacc, mybir
from concourse.bass_utils import run_bass_kernel_spmd

# Problem shape (hardcoded per spec)
B = 64          # batch
P = 4096        # pixel_number
KPP = 64        # kernels_per_pixel
CKS = 288       # C * kernel_size
NCORES = 8
PPC = P // NCORES          # 512 pixels per core
KC = 128                   # main contraction chunk rows
KT = CKS - 2 * KC          # 32 tail rows
PX = 64                    # pixels per SBUF tile
NT = PPC // PX             # 8 pixel tiles per core
NPK = NT // 4              # G-tail packs (4 tiles per pack)
GRP = 16                   # pixels per PSUM bank tile (2 x 8 pairs)

BF16 = mybir.dt.bfloat16
F32 = mybir.dt.float32

_IN_DT = os.environ.get("KERNEL_IN_DT", "fp8e3")
if _IN_DT == "fp8e3":
    SCALE = 2.0            # fp8 pre-scale per operand (unscale on host)
    FP8 = mybir.dt.float8e3
    NP_FP8 = ml_dtypes.float8_e3m4
else:  # bf16
    SCALE = 1.0
    FP8 = mybir.dt.bfloat16
    NP_FP8 = ml_dtypes.bfloat16

_NC_CACHE = {}


def _build_nc():
    if "nc" in _NC_CACHE:
        return _NC_CACHE["nc"]
    nc = bacc.Bacc(None, target_bir_lowering=False)

    # both K=128 main chunks merged per tile: one large-descriptor DMA each
    g_par = nc.declare_dram_parameter("g", [KC, 2 * PPC * B], FP8, isOutput=False)
    w_par = nc.declare_dram_parameter("w", [KC, 2 * PPC * KPP], FP8, isOutput=False)
    # g tails packed 4-up into 128 partitions (band t%4 = tile t, pack t//4)
    g2_par = nc.declare_dram_parameter(
        "g2", [4 * KT, NPK * PX * B], FP8, isOutput=False
    )
    # w tails, thin layout [32, P*KPP], banded into zeroed slots on device
    w2_par = nc.declare_dram_parameter("w2", [KT, PPC * KPP], FP8, isOutput=False)
    out_par = nc.declare_dram_parameter(
        "out", [2 * KPP, (PPC // 2) * B], BF16, isOutput=True
    )

    with tile.TileContext(nc) as tc:
        with (
            tc.tile_pool(name="gio", bufs=4) as gio,
            tc.tile_pool(name="wio", bufs=4) as wio,
            tc.tile_pool(name="oio", bufs=3) as oio,
            tc.tile_pool(name="ext", bufs=1) as ext,
            tc.tile_pool(name="ps", bufs=8, space="PSUM") as ps_pool,
        ):
            # G tails: full-width 4-up packs, band t%4 holds tile t's 32
            # rows; non-band rows carry other tiles' data, nulled by the
            # zero rows of the W-tail slot in the K=128 tail matmul.
            egs = []
            for i in range(NPK):
                eg = ext.tile([4 * KT, PX * B], FP8, tag=f"eg{i}")
                nc.sync.dma_start(
                    out=eg[:, :], in_=g2_par[:, i * PX * B : (i + 1) * PX * B]
                )
                egs.append(eg)
            # W tails: 4 band slots; quadrants off the band zeroed once
            # up-front (merged runs, never overlapping the band DMA)
            ews = []
            _ZRUNS = {
                0: [(32, 64), (64, 128)],
                1: [(0, 32), (64, 128)],
                2: [(0, 64), (96, 128)],
                3: [(0, 96)],
            }
            for band in range(4):
                ew = ext.tile([4 * KT, PX * KPP], FP8, tag=f"ew{band}")
                for lo, hi in _ZRUNS[band]:
                    nc.gpsimd.memset(ew[lo:hi, :], 0.0)
                ews.append(ew)
            for t in range(NT):
                ocols = slice(t * (PX // 2) * B, (t + 1) * (PX // 2) * B)
                band = t % 4
                bs = slice(band * KT, (band + 1) * KT)
                ew = ews[band]
                eg = egs[t // 4]
                nc.scalar.dma_start(
                    out=ew[bs, :],
                    in_=w2_par[:, t * PX * KPP : (t + 1) * PX * KPP],
                )
                gm = gio.tile([KC, 2 * PX * B], FP8, tag="g")
                nc.sync.dma_start(
                    out=gm[:, :],
                    in_=g_par[:, t * 2 * PX * B : (t + 1) * 2 * PX * B],
                )
                wm = wio.tile([KC, 2 * PX * KPP], FP8, tag="w")
                nc.scalar.dma_start(
                    out=wm[:, :],
                    in_=w_par[:, t * 2 * PX * KPP : (t + 1) * 2 * PX * KPP],
                )
                g_t = [gm[:, : PX * B], gm[:, PX * B :], eg]
                w_t = [wm[:, : PX * KPP], wm[:, PX * KPP :], ew]
                o_t = oio.tile([2 * KPP, (PX // 2) * B], BF16, tag="o")
                for grp in range(PX // GRP):
                    # [128, 512] PSUM tile: even pixel of each pair in
                    # partitions 0-63 (PE col-tile T0), odd in 64-127 (T1).
                    ps = ps_pool.tile([2 * KPP, (GRP // 2) * B],
                                      mybir.dt.float32, tag="ps")
                    for q in range(GRP):
                        lp = (grp * GRP + q) * B
                        lpk = (grp * GRP + q) * KPP
                        half = q % 2
                        prow = slice(half * KPP, (half + 1) * KPP)
                        pcol = slice((q // 2) * B, (q // 2 + 1) * B)
                        for j in range(3):
                            nc.tensor.matmul(
                                ps[prow, pcol],
                                w_t[j][:, lpk : lpk + KPP],
                                g_t[j][:, lp : lp + B],
                                start=(j == 0),
                                stop=(j == 2),
                                tile_position=(0, half * KPP),
                            )
                    # o_t rows: even pixel k in partitions 0-63, odd in
                    # 64-127; col = pair_idx * B + b (unscrambled on host).
                    ob = slice(grp * (GRP // 2) * B, (grp + 1) * (GRP // 2) * B)
                    if grp % 2 == 0:
                        nc.vector.tensor_copy(o_t[:, ob], ps[:, :])
                    else:
                        nc.scalar.copy(o_t[:, ob], ps[:, :])
                hw_ = (PX // 4) * B
                for hh in range(2):
                    hs = slice(hh * hw_, (hh + 1) * hw_)
                    ds = slice(t * (PX // 2) * B + hh * hw_,
                               t * (PX // 2) * B + (hh + 1) * hw_)
                    nc.gpsimd.dma_start(out=out_par[:, ds], in_=o_t[:, hs])
    nc.compile()
    _NC_CACHE["nc"] = nc
    return nc


def _prepare_in_maps(x, hashtable, weights):
    x = np.ascontiguousarray(np.asarray(x), dtype=np.float32)
    hashtable = np.asarray(hashtable)
    weights = np.asarray(weights, dtype=np.float32)

    # Hash-indexed regrouping of image values per pixel (data layout only).
    gathered = x.reshape(-1)[hashtable[: P * B]]            # (B*P, CKS) f32
    g_q = (gathered * SCALE).astype(NP_FP8)
    g_cpb = g_q.reshape(B, P, CKS).transpose(2, 1, 0)       # (CKS, P, B)

    w_q = (weights * SCALE).astype(NP_FP8)
    w_cpk = w_q.transpose(2, 0, 1)                          # (CKS, P, KPP)

    def tail_pack4(src, pix, d):
        # (KT, PPC, d) -> [4*KT, NPK*PX*d]: pack i = tiles 4i..4i+3, band
        # rows 32*(t%4)..+32 = tile t's tail over its PX pixels
        a = src[2 * KC :, pix, :]                            # (KT, PPC, d)
        a = a.reshape(KT, NPK, 4, PX, d)                     # (c, i, band, p, d)
        a = a.transpose(2, 0, 1, 3, 4)                       # (band, c, i, p, d)
        return np.ascontiguousarray(a).reshape(4 * KT, NPK * PX * d)

    def tail_thin(src, pix, d):
        a = src[2 * KC :, pix, :]                            # (KT, PPC, d)
        return np.ascontiguousarray(a).reshape(KT, PPC * d)

    def main_merge(src, pix, d):
        # (2*KC, PPC, d) -> [KC, NT*2*PX*d]: per pixel tile, chunk0 block
        # then chunk1 block
        a = src[: 2 * KC, pix, :]                            # (256, PPC, d)
        a = a.reshape(2, KC, NT, PX, d)                      # (j, c, t, p, d)
        a = a.transpose(1, 2, 0, 3, 4)                       # (c, t, j, p, d)
        return np.ascontiguousarray(a).reshape(KC, 2 * PPC * d)

    in_maps = []
    for i in range(NCORES):
        pix = slice(i * PPC, (i + 1) * PPC)
        m = {
            "g": main_merge(g_cpb, pix, B),
            "w": main_merge(w_cpk, pix, KPP),
            "g2": tail_pack4(g_cpb, pix, B),
            "w2": tail_thin(w_cpk, pix, KPP),
        }
        in_maps.append(m)
    return in_maps


def _assemble(results):
    out = np.empty((B, KPP, P), dtype=np.float32)
    inv = 1.0 / (SCALE * SCALE)
    for i in range(NCORES):
        o = np.asarray(results[i]["out"]).astype(np.float32)
        o = o.reshape(2, KPP, PPC // 2, B)                  # (half, k, p2, b)
        out[:, :, i * PPC : (i + 1) * PPC] = o.transpose(3, 1, 2, 0).reshape(
            B, KPP, PPC
        ) * inv
    return out


def run(x, hashtable, weights, trace=False):
    nc = _build_nc()
    in_maps = _prepare_in_maps(x, hashtable, weights)
    res = run_bass_kernel_spmd(
        nc, in_maps, core_ids=list(range(NCORES)), trace=trace
    )
    return _assemble(res.results), res


def kernel(x, hashtable, weights):
    out, _ = run(x, hashtable, weights, trace=False)
    return out
